# revision 9
# baseline (speedup 1.0000x reference)
"""Single-head causal attention (B=4, S=4096, D_IN=256, D_OUT=64) on 8 TRN2 cores.

Sharding (SPMD, one Bass program, per-core data):
  - 2 cores per batch. Per batch, the 8 query blocks of 512 rows split by causal
    workload: member A (core%2==0) takes odd blocks {1,3,5,7} (k-chunk counts
    8,16,24,32), member B takes even blocks {0,2,4,6} (counts 4,12,20,28, padded
    +4 junk chunks each so every core runs the identical program).
  - Program: 4 q-slots of 512 rows; slot s iterates C_s = 8(s+1) k-chunks of
    128, in groups of 2. The last 8 k-chunk positions of each slot are
    multiplied by per-core mask tiles (A: [1,1,1,1,M1..M4], B: [M1..M4,0,0,0,0])
    implementing the causal mask and neutralizing B's padding.
  - Layouts: host passes X^T (d_in on partitions). Q^T/K^T [64, seq] from
    lhsT=W chunks (f32r, strip-duplicated to partitions 64:128 for PE row-tile
    packing); V natural [k,64] from lhsT=Xv^T chunks (bf16, FWL); S^T group =
    two [64,128]^T @ [64,512] f32r matmuls into one [128,1024] PSUM tile; exp
    on ACT into bf16 P^T; PV accumulates lhsT=V'[k,65] (ones column fuses the
    softmax row-sum) into PSUM [65,512]. PE transpose -> [q,65], divide by
    column 64, batched DMA out.
"""

import numpy as np
import ml_dtypes

B, S, D_IN, D_OUT = 4, 4096, 256, 64
N_CORES = 8
QS = 512            # q rows per slot
N_SLOTS = 4         # slots per core
KC = 128            # k rows per chunk
QT = QS * N_SLOTS   # 2048 q rows per core
STRIPS = 2          # QK row-strip packing (1 or 2)

_STATE = {}


def _build_program(repeats=1):
    from contextlib import ExitStack
    import concourse.tile as tile
    from concourse import bacc, mybir
    import concourse.bass as bass
    ts = bass.ts

    f32 = mybir.dt.float32
    f32r = mybir.dt.float32r
    bf16 = mybir.dt.bfloat16
    Exp = mybir.ActivationFunctionType.Exp

    nc = bacc.Bacc("TRN2", target_bir_lowering=False, debug=False,
                   num_devices=N_CORES)

    xq = nc.dram_tensor("xq_t", [D_IN, QT], f32r, kind="ExternalInput").ap()
    xk = nc.dram_tensor("xk_t", [D_IN, S], f32r, kind="ExternalInput").ap()
    xv = nc.dram_tensor("xv_t", [D_IN, S], f32, kind="ExternalInput").ap()
    wq = nc.dram_tensor("wq", [D_IN, D_OUT], f32r, kind="ExternalInput").ap()
    wk = nc.dram_tensor("wk", [D_IN, D_OUT], f32r, kind="ExternalInput").ap()
    wv = nc.dram_tensor("wv", [D_IN, D_OUT], f32, kind="ExternalInput").ap()
    masks = nc.dram_tensor("masks", [128, 8 * QS], bf16, kind="ExternalInput").ap()
    ident = nc.dram_tensor("ident", [128, 128], f32, kind="ExternalInput").ap()
    out = nc.dram_tensor("out", [QT, D_OUT], f32, kind="ExternalOutput").ap()

    xq_r = xq.rearrange("(c p) n -> p c n", p=128)
    xk_r = xk.rearrange("(c p) n -> p c n", p=128)
    xv_r = xv.rearrange("(c p) n -> p c n", p=128)

    with tile.TileContext(nc) as tc:
        with ExitStack() as ctx:
            const = ctx.enter_context(tc.tile_pool(name="const", bufs=1))
            xin = ctx.enter_context(tc.tile_pool(name="xin", bufs=1))
            kt_pool = ctx.enter_context(tc.tile_pool(name="ktp", bufs=8))
            qt_pool = ctx.enter_context(tc.tile_pool(name="qtp", bufs=4))
            vp_pool = ctx.enter_context(tc.tile_pool(name="vpp", bufs=8))
            pt_pool = ctx.enter_context(tc.tile_pool(name="ptp", bufs=20))
            o_pool = ctx.enter_context(tc.tile_pool(name="op", bufs=2))
            ob_pool = ctx.enter_context(tc.tile_pool(name="obp", bufs=2))
            rc_pool = ctx.enter_context(tc.tile_pool(name="rcp", bufs=4))
            ps_a = ctx.enter_context(tc.tile_pool(name="ps_a", space="PSUM", bufs=2))
            ps_b = ctx.enter_context(tc.tile_pool(name="ps_b", space="PSUM", bufs=2))
            ps_o = ctx.enter_context(tc.tile_pool(name="ps_o", space="PSUM", bufs=2))

            # ---- constants ----
            wq_sb = const.tile([128, 2, D_OUT], f32r, tag="wq")
            nc.sync.dma_start(wq_sb[:], wq.rearrange("(c p) d -> p c d", p=128))
            wk_sb = const.tile([128, 2, D_OUT], f32r, tag="wk")
            nc.sync.dma_start(wk_sb[:], wk.rearrange("(c p) d -> p c d", p=128))
            wv_sb = const.tile([128, 2, D_OUT], bf16, tag="wv")
            nc.gpsimd.dma_start(wv_sb[:], wv.rearrange("(c p) d -> p c d", p=128))
            mask_sb = const.tile([128, 8 * QS], bf16, tag="masks")
            nc.sync.dma_start(mask_sb[:], masks[:])
            id_sb = const.tile([128, 128], f32, tag="ident")
            nc.sync.dma_start(id_sb[:], ident[:])

            def body():
                # ---- chunked input loads (512 seq-cols per DMA) ----
                xq_t = [xin.tile([128, 2, QS], f32r, tag=f"xq{t}", name=f"xq{t}")
                        for t in range(4)]
                xk_t = [xin.tile([128, 2, QS], f32r, tag=f"xk{t}", name=f"xk{t}")
                        for t in range(8)]
                xv_t = [xin.tile([128, 2, QS], bf16, tag=f"xv{t}", name=f"xv{t}")
                        for t in range(8)]
                for t in range(8):
                    nc.sync.dma_start(xk_t[t][:], xk_r[:, :, ts(t, QS)])
                    nc.gpsimd.dma_start(xv_t[t][:], xv_r[:, :, ts(t, QS)])
                    if t < 4:
                        nc.sync.dma_start(xq_t[t][:], xq_r[:, :, ts(t, QS)])

                kt_tiles = [None] * 8
                qt_tiles = [None] * 4
                vp_tiles = [None] * 8

                def _wx_proj(w_sb, x_tile, pool, tg):
                    # project into strip 0 (partitions 0:64); with STRIPS=2
                    # also into strip 1 (64:128) via twin col-packed matmuls
                    ps = ps_b.tile([128, QS], f32, tag="ps_b", name="ps")
                    nc.tensor.matmul(ps[0:64, :], w_sb[:, 0, :],
                                     x_tile[:, 0, :], start=True, stop=False)
                    nc.tensor.matmul(ps[0:64, :], w_sb[:, 1, :],
                                     x_tile[:, 1, :], start=False, stop=True)
                    res = pool.tile([128, QS], f32r, tag=tg, name=tg)
                    nc.vector.tensor_copy(res[0:64, :], ps[0:64, :])
                    if STRIPS == 2:
                        nc.gpsimd.tensor_copy(res[64:128, :], res[0:64, :])
                    return res

                def k_proj(t):
                    kt_tiles[t] = _wx_proj(wk_sb, xk_t[t], kt_pool, "kt")

                def q_proj(t):
                    qt_tiles[t] = _wx_proj(wq_sb, xq_t[t], qt_pool, "qt")

                def v_proj(t):  # V' tiles for k-tiles 4t..4t+3
                    ps = ps_b.tile([128, 4, D_OUT], f32, tag="ps_b")
                    for u in range(4):
                        nc.tensor.matmul(ps[:, u, :], xv_t[t][:, 0, ts(u, KC)],
                                         wv_sb[:, 0, :], start=True, stop=False)
                        nc.tensor.matmul(ps[:, u, :], xv_t[t][:, 1, ts(u, KC)],
                                         wv_sb[:, 1, :], start=False, stop=True)
                    vp = vp_pool.tile([128, 4, D_OUT + 1], bf16, tag="vp")
                    nc.vector.memset(vp[:, :, D_OUT:D_OUT + 1], 1.0)
                    nc.vector.tensor_copy(vp[:, :, 0:D_OUT], ps[:])
                    vp_tiles[t] = vp

                prev = None  # (pv closures, epilogue closure) of previous slot

                def slot(s):
                    nonlocal prev
                    cg = 4 * (s + 1)          # groups of 2 k-chunks
                    qtile = qt_tiles[s]
                    po = ps_o.tile([D_OUT + 1, QS], f32, tag="ps_o")
                    pvs = []

                    def make_pv(g, pt):
                        def emit():
                            for c in range(2):
                                j = 2 * g + c
                                nc.tensor.matmul(
                                    po[:], vp_tiles[j // 4][:, j % 4, :],
                                    pt[:, c, :],
                                    start=(j == 0), stop=(j == 2 * cg - 1))
                        return emit

                    def make_epi():
                        def emit():
                            osb = o_pool.tile([D_OUT + 1, QS], f32, tag="osb")
                            nc.vector.tensor_copy(osb[:], po[:])
                            ob = ob_pool.tile([128, 4, D_OUT], f32, tag="ob")
                            for t2 in range(4):
                                pst = ps_b.tile([128, D_OUT + 1], f32, tag="ps_b")
                                nc.tensor.transpose(
                                    pst[:], osb[:, ts(t2, 128)],
                                    id_sb[0:D_OUT + 1, 0:D_OUT + 1])
                                rc = rc_pool.tile([128, 1], f32, tag="rc")
                                nc.vector.reciprocal(rc[:], pst[:, D_OUT:D_OUT + 1])
                                nc.vector.tensor_scalar_mul(
                                    ob[:, t2, :], pst[:, 0:D_OUT], rc[:])
                            r0 = QS * s
                            nc.gpsimd.dma_start(
                                out[r0:r0 + QS, :].rearrange(
                                    "(t p) d -> p t d", p=128),
                                ob[:])
                        return emit

                    for g in range(cg):
                        pss = ps_a.tile([128, 2, QS], f32, tag="ps_a")
                        for c in range(2):
                            j = 2 * g + c
                            par = (j % 2) * 64 if STRIPS == 2 else 0
                            nc.tensor.matmul(
                                pss[:, c, :],
                                kt_tiles[j // 4][par:par + 64, ts(j % 4, KC)],
                                qtile[par:par + 64, :],
                                start=True, stop=True)
                        pt = pt_pool.tile([128, 2, QS], bf16, tag="pt")
                        nc.scalar.activation(pt[:], pss[:], Exp)
                        if g >= cg - 4:
                            m = g - (cg - 4)
                            nc.vector.tensor_mul(
                                pt[:], pt[:],
                                mask_sb[:, 2 * QS * m:2 * QS * (m + 1)].rearrange(
                                    "p (c n) -> p c n", c=2))
                        pvs.append(make_pv(g, pt))
                        if prev is not None and prev[0]:
                            prev[0].pop(0)()
                    if prev is not None:
                        while prev[0]:
                            prev[0].pop(0)()
                        prev[1]()
                    prev = (pvs, make_epi())

                for s in range(N_SLOTS):
                    for t in (2 * s, 2 * s + 1):
                        k_proj(t)
                        v_proj(t)
                    q_proj(s)
                    slot(s)
                while prev[0]:
                    prev[0].pop(0)()
                prev[1]()

            for _rep in range(repeats):
                body()

    nc.compile()
    return nc


def _host_inputs(inputs):
    """Build the 8 per-core input maps."""
    xq_full = np.asarray(inputs["inputs_for_queries"], dtype=np.float32)
    xk_full = np.asarray(inputs["inputs_for_keys"], dtype=np.float32)
    xv_full = np.asarray(inputs["inputs_for_values"], dtype=np.float32)
    wq = np.asarray(inputs["wq"], dtype=np.float32) / np.sqrt(np.float32(D_OUT))
    wk = np.asarray(inputs["wk"], dtype=np.float32)
    wv = np.asarray(inputs["wv"], dtype=np.float32)

    dk = np.arange(128, dtype=np.int64)[:, None]
    dq = np.arange(QS, dtype=np.int64)[None, :]
    mtiles = [(dk + 128 * i <= dq).astype(np.float32) for i in range(4)]
    ones = np.ones((128, QS), np.float32)
    zeros = np.zeros((128, QS), np.float32)
    mask_a = np.concatenate([ones] * 4 + mtiles, 1).astype(ml_dtypes.bfloat16)
    mask_b = np.concatenate(mtiles + [zeros] * 4, 1).astype(ml_dtypes.bfloat16)
    ident = np.eye(128, dtype=np.float32)

    in_maps = []
    for c in range(N_CORES):
        b, m = divmod(c, 2)
        blocks = [2 * s + 1 - m for s in range(N_SLOTS)]
        qsel = np.concatenate([xq_full[b, QS * i:QS * i + QS, :] for i in blocks], 0)
        in_maps.append({
            "xq_t": np.ascontiguousarray(qsel.T),
            "xk_t": np.ascontiguousarray(xk_full[b].T),
            "xv_t": np.ascontiguousarray(xv_full[b].T),
            "wq": wq, "wk": wk, "wv": wv,
            "masks": mask_b if m else mask_a,
            "ident": ident,
        })
    return in_maps


def _assemble(results):
    out = np.empty((B, S, D_OUT), dtype=np.float32)
    for c in range(N_CORES):
        b, m = divmod(c, 2)
        co = results[c]["out"]
        for s in range(N_SLOTS):
            i = 2 * s + 1 - m
            out[b, QS * i:QS * i + QS, :] = co[QS * s:QS * s + QS, :]
    return out


def _run(inputs, trace=False):
    from concourse.bass_utils import run_bass_kernel_spmd
    if "nc" not in _STATE:
        _STATE["nc"] = _build_program()
    res = run_bass_kernel_spmd(_STATE["nc"], _host_inputs(inputs),
                               list(range(N_CORES)), trace=trace)
    return _assemble(res.results), res


def kernel(**inputs):
    out, _ = _run(inputs, trace=False)
    return out


# revision 20
# speedup vs baseline: 18.0398x; 18.0398x over previous
"""Single-head causal attention (B=4, S=4096, D_IN=256, D_OUT=64) on 8 TRN2 cores.

Sharding (SPMD, one Bass program, per-core data):
  - 2 cores per batch. Per batch, the query blocks of QS rows are split by
    causal workload: member A (core%2==0) takes odd blocks, member B even
    blocks (B's per-slot k-chunk count is NMSK/2 short and padded with junk
    chunks so every core runs the identical program; masks zero the junk).
  - Program: N_SLOTS q-slots of QS rows; slot s iterates ck = (s+1)*NMSK
    k-chunks of 128, fused into exp groups of GRP chunks. The last NMSK
    k-chunk positions of each slot are multiplied by per-core mask tiles
    (A: [1]*NMSK/2 + [M1..], B: [M1..] + [0]*NMSK/2) implementing the causal
    mask and neutralizing the padding.
  - Layouts: host passes X^T (d_in on partitions). Q^T/K^T [64, seq] from
    lhsT=W chunks (f32r), duplicated to partitions 64:128 so consecutive QK
    matmuls run in disjoint PE row-tiles (measured ~1.5x); V natural [k,64]
    from lhsT=Xv^T chunks (bf16, FWL); S^T group = GRP [64,128]^T @ [64,QS]
    f32r matmuls into one [128,GRP*QS] PSUM tile; exp on ACT into bf16 P^T;
    PV accumulates lhsT=V'[k,65] (ones column fuses the softmax row-sum) into
    PSUM [65,QS]. PV of slot s is interleaved into slot s+1's QK stream.
    Epilogue: PE transpose -> [q,65], multiply by reciprocal of column 64,
    batched DMA out.
"""

import numpy as np
import ml_dtypes

B, S, D_IN, D_OUT = 4, 4096, 256, 64
N_CORES = 8
QS = 256            # q rows per slot
KC = 128            # k rows per chunk
QT = 2048           # q rows per core
N_SLOTS = QT // QS  # slots per core
GRP = 1024 // QS    # k-chunks fused per exp group
NMSK = QS // 64     # masked k-chunk positions per slot
STRIPS = 2          # QK row-strip packing (1 or 2)
# chunk c of a group is stored at psum/pt slice PERM[c]; for GRP=4 this puts
# concurrently-running strip pairs (c even/odd) in different PSUM banks
PERM = [0, 2, 1, 3] if GRP == 4 else list(range(GRP))
IN_BF16 = True      # host passes bf16 X^T inputs (halves HBM traffic)

_STATE = {}


def _build_program(repeats=1):
    from contextlib import ExitStack
    import concourse.tile as tile
    from concourse import bacc, mybir
    import concourse.bass as bass
    ts = bass.ts

    f32 = mybir.dt.float32
    f32r = mybir.dt.float32r
    bf16 = mybir.dt.bfloat16
    Exp = mybir.ActivationFunctionType.Exp

    nc = bacc.Bacc("TRN2", target_bir_lowering=False, debug=False,
                   num_devices=N_CORES)

    xdt = bf16 if IN_BF16 else f32r
    xq = nc.dram_tensor("xq_t", [D_IN, QT], xdt, kind="ExternalInput").ap()
    xk = nc.dram_tensor("xk_t", [D_IN, S], xdt, kind="ExternalInput").ap()
    xv = nc.dram_tensor("xv_t", [D_IN, S], bf16 if IN_BF16 else f32,
                        kind="ExternalInput").ap()
    wq = nc.dram_tensor("wq", [D_IN, D_OUT], f32r, kind="ExternalInput").ap()
    wk = nc.dram_tensor("wk", [D_IN, D_OUT], f32r, kind="ExternalInput").ap()
    wv = nc.dram_tensor("wv", [D_IN, D_OUT], f32, kind="ExternalInput").ap()
    masks = nc.dram_tensor("masks", [128, NMSK * QS], bf16,
                           kind="ExternalInput").ap()
    ident = nc.dram_tensor("ident", [128, 128], f32, kind="ExternalInput").ap()
    out = nc.dram_tensor("out", [QT, D_OUT], f32, kind="ExternalOutput").ap()

    xq_r = xq.rearrange("(c p) n -> p c n", p=128)
    xk_r = xk.rearrange("(c p) n -> p c n", p=128)
    xv_r = xv.rearrange("(c p) n -> p c n", p=128)

    with tile.TileContext(nc) as tc:
        with ExitStack() as ctx:
            const = ctx.enter_context(tc.tile_pool(name="const", bufs=1))
            xin = ctx.enter_context(tc.tile_pool(name="xin", bufs=1))
            kt_pool = ctx.enter_context(tc.tile_pool(name="ktp", bufs=8))
            qt_pool = ctx.enter_context(tc.tile_pool(name="qtp", bufs=4))
            vp_pool = ctx.enter_context(tc.tile_pool(name="vpp", bufs=8))
            pt_pool = ctx.enter_context(tc.tile_pool(name="ptp", bufs=16))
            o_pool = ctx.enter_context(tc.tile_pool(name="op", bufs=2))
            ob_pool = ctx.enter_context(tc.tile_pool(name="obp", bufs=2))
            rc_pool = ctx.enter_context(tc.tile_pool(name="rcp", bufs=4))
            ps_a = ctx.enter_context(tc.tile_pool(name="ps_a", space="PSUM", bufs=2))
            ps_b = ctx.enter_context(tc.tile_pool(name="ps_b", space="PSUM", bufs=2))
            ps_o = ctx.enter_context(tc.tile_pool(name="ps_o", space="PSUM", bufs=2))

            # ---- constants ----
            wdt = bf16 if IN_BF16 else f32r
            weng = nc.gpsimd if IN_BF16 else nc.sync
            wq_sb = const.tile([128, 2, D_OUT], wdt, tag="wq")
            weng.dma_start(wq_sb[:], wq.rearrange("(c p) d -> p c d", p=128))
            wk_sb = const.tile([128, 2, D_OUT], wdt, tag="wk")
            weng.dma_start(wk_sb[:], wk.rearrange("(c p) d -> p c d", p=128))
            wv_sb = const.tile([128, 2, D_OUT], bf16, tag="wv")
            nc.gpsimd.dma_start(wv_sb[:], wv.rearrange("(c p) d -> p c d", p=128))
            mask_sb = const.tile([128, NMSK * QS], bf16, tag="masks")
            nc.sync.dma_start(mask_sb[:], masks[:])
            id_sb = const.tile([128, 128], f32, tag="ident")
            nc.sync.dma_start(id_sb[:], ident[:])

            def body():
                # ---- chunked input loads (512 seq-cols per DMA) ----
                xdt_s = bf16 if IN_BF16 else f32r
                xq_t = [xin.tile([128, 2, 512], xdt_s, tag=f"xq{t}", name=f"xq{t}")
                        for t in range(4)]
                xk_t = [xin.tile([128, 2, 512], xdt_s, tag=f"xk{t}", name=f"xk{t}")
                        for t in range(8)]
                xv_t = [xin.tile([128, 2, 512], bf16, tag=f"xv{t}", name=f"xv{t}")
                        for t in range(8)]
                xveng = nc.sync if IN_BF16 else nc.gpsimd
                for t in range(8):
                    nc.sync.dma_start(xk_t[t][:], xk_r[:, :, ts(t, 512)])
                    xveng.dma_start(xv_t[t][:], xv_r[:, :, ts(t, 512)])
                    if t < 4:
                        nc.sync.dma_start(xq_t[t][:], xq_r[:, :, ts(t, 512)])

                kt_tiles = [None] * 8
                qt_tiles = [None] * 4
                vp_tiles = [None] * 8

                def _wx_proj(w_sb, x_tile, pool, tg):
                    ps = ps_b.tile([128, 512], f32, tag="ps_b", name="ps")
                    nc.tensor.matmul(ps[0:64, :], w_sb[:, 0, :],
                                     x_tile[:, 0, :], start=True, stop=False)
                    nc.tensor.matmul(ps[0:64, :], w_sb[:, 1, :],
                                     x_tile[:, 1, :], start=False, stop=True)
                    res = pool.tile([128, 512], bf16 if IN_BF16 else f32r,
                                    tag=tg, name=tg)
                    nc.vector.tensor_copy(res[0:64, :], ps[0:64, :])
                    if STRIPS == 2:
                        nc.vector.tensor_copy(res[64:128, :], ps[0:64, :])
                    return res

                def k_proj(t):
                    kt_tiles[t] = _wx_proj(wk_sb, xk_t[t], kt_pool, "kt")

                def q_proj(t):
                    qt_tiles[t] = _wx_proj(wq_sb, xq_t[t], qt_pool, "qt")

                def v_proj(t):  # V' tiles for k-tiles 4t..4t+3
                    ps = ps_b.tile([128, 4, D_OUT], f32, tag="ps_b")
                    for u in range(4):
                        nc.tensor.matmul(ps[:, u, :], xv_t[t][:, 0, ts(u, KC)],
                                         wv_sb[:, 0, :], start=True, stop=False)
                        nc.tensor.matmul(ps[:, u, :], xv_t[t][:, 1, ts(u, KC)],
                                         wv_sb[:, 1, :], start=False, stop=True)
                    vp = vp_pool.tile([128, 4, D_OUT + 1], bf16, tag="vp")
                    nc.vector.memset(vp[:, :, D_OUT:D_OUT + 1], 1.0)
                    nc.vector.tensor_copy(vp[:, :, 0:D_OUT], ps[:])
                    vp_tiles[t] = vp

                prev = None  # (pv closures, epilogue closure) of previous slot

                def slot(s):
                    nonlocal prev
                    ck = (s + 1) * NMSK       # k-chunks this slot
                    cg = ck // GRP            # exp groups
                    qtile = qt_tiles[(s * QS) // 512]
                    qoff = (s * QS) % 512
                    po = ps_o.tile([D_OUT + 1, QS], f32, tag="ps_o")
                    pvs = []

                    def make_pv(g, pt):
                        def emit():
                            for c in range(GRP):
                                j = GRP * g + c
                                nc.tensor.matmul(
                                    po[:], vp_tiles[j // 4][:, j % 4, :],
                                    pt[:, PERM[c], :],
                                    start=(j == 0), stop=(j == ck - 1))
                        return emit

                    def make_epi():
                        def emit():
                            osb = o_pool.tile([D_OUT + 1, QS], f32, tag="osb")
                            nc.vector.tensor_copy(osb[:], po[:])
                            nt = QS // 128
                            ob = ob_pool.tile([128, nt, D_OUT], f32, tag="ob")
                            for t2 in range(nt):
                                pst = ps_b.tile([128, D_OUT + 1], f32, tag="ps_b")
                                nc.tensor.transpose(
                                    pst[:], osb[:, ts(t2, 128)],
                                    id_sb[0:D_OUT + 1, 0:D_OUT + 1])
                                rc = rc_pool.tile([128, 1], f32, tag="rc")
                                nc.vector.reciprocal(rc[:], pst[:, D_OUT:D_OUT + 1])
                                nc.vector.tensor_scalar_mul(
                                    ob[:, t2, :], pst[:, 0:D_OUT], rc[:])
                            r0 = QS * s
                            nc.gpsimd.dma_start(
                                out[r0:r0 + QS, :].rearrange(
                                    "(t p) d -> p t d", p=128),
                                ob[:])
                        return emit

                    nmg = NMSK // GRP         # masked groups (last nmg)
                    for g in range(cg):
                        pss = ps_a.tile([128, GRP, QS], f32, tag="ps_a")
                        for c in range(GRP):
                            j = GRP * g + c
                            par = (j % 2) * 64 if STRIPS == 2 else 0
                            nc.tensor.matmul(
                                pss[:, PERM[c], :],
                                kt_tiles[j // 4][par:par + 64, ts(j % 4, KC)],
                                qtile[par:par + 64, qoff:qoff + QS],
                                start=True, stop=True)
                        pt = pt_pool.tile([128, GRP, QS], bf16, tag="pt")
                        nc.scalar.activation(pt[:], pss[:], Exp)
                        if g >= cg - nmg:
                            m = g - (cg - nmg)
                            nc.vector.tensor_mul(
                                pt[:], pt[:],
                                mask_sb[:, GRP * QS * m:GRP * QS * (m + 1)]
                                .rearrange("p (c n) -> p c n", c=GRP))
                        pvs.append(make_pv(g, pt))
                        if prev is not None and prev[0]:
                            prev[0].pop(0)()
                    if prev is not None:
                        while prev[0]:
                            prev[0].pop(0)()
                        prev[1]()
                    prev = (pvs, make_epi())

                done_kt = 0
                done_qt = 0
                for s in range(N_SLOTS):
                    need_kt = ((s + 1) * NMSK - 1) // 4 + 1
                    while done_kt < need_kt:
                        k_proj(done_kt)
                        v_proj(done_kt)
                        done_kt += 1
                    need_qt = (s * QS) // 512 + 1
                    while done_qt < need_qt:
                        q_proj(done_qt)
                        done_qt += 1
                    slot(s)
                while prev[0]:
                    prev[0].pop(0)()
                prev[1]()

            for _rep in range(repeats):
                body()

    nc.compile()
    return nc


def _host_inputs(inputs):
    """Build the 8 per-core input maps."""
    xq_full = np.asarray(inputs["inputs_for_queries"], dtype=np.float32)
    xk_full = np.asarray(inputs["inputs_for_keys"], dtype=np.float32)
    xv_full = np.asarray(inputs["inputs_for_values"], dtype=np.float32)
    wq = np.asarray(inputs["wq"], dtype=np.float32) / np.sqrt(np.float32(D_OUT))
    wk = np.asarray(inputs["wk"], dtype=np.float32)
    wv = np.asarray(inputs["wv"], dtype=np.float32)

    dk = np.arange(128, dtype=np.int64)[:, None]
    dq = np.arange(QS, dtype=np.int64)[None, :]
    nh = NMSK // 2
    mtiles = [(dk + 128 * i <= dq).astype(np.float32) for i in range(nh)]
    ones = np.ones((128, QS), np.float32)
    zeros = np.zeros((128, QS), np.float32)
    pos_a = [ones] * nh + mtiles
    pos_b = mtiles + [zeros] * nh
    # mask slice sp multiplies the chunk stored there (PERM is an involution)
    arr_a = [None] * NMSK
    arr_b = [None] * NMSK
    for g in range(NMSK // GRP):
        for c in range(GRP):
            arr_a[g * GRP + PERM[c]] = pos_a[g * GRP + c]
            arr_b[g * GRP + PERM[c]] = pos_b[g * GRP + c]
    mask_a = np.concatenate(arr_a, 1).astype(ml_dtypes.bfloat16)
    mask_b = np.concatenate(arr_b, 1).astype(ml_dtypes.bfloat16)
    ident = np.eye(128, dtype=np.float32)

    in_maps = []
    for c in range(N_CORES):
        b, m = divmod(c, 2)
        blocks = [2 * s + 1 - m for s in range(N_SLOTS)]
        qsel = np.concatenate([xq_full[b, QS * i:QS * i + QS, :] for i in blocks], 0)
        cast = (lambda a: a.astype(ml_dtypes.bfloat16)) if IN_BF16 else (lambda a: a)
        in_maps.append({
            "xq_t": cast(np.ascontiguousarray(qsel.T)),
            "xk_t": cast(np.ascontiguousarray(xk_full[b].T)),
            "xv_t": cast(np.ascontiguousarray(xv_full[b].T)),
            "wq": wq, "wk": wk, "wv": wv,
            "masks": mask_b if m else mask_a,
            "ident": ident,
        })
    return in_maps


def _assemble(results):
    out = np.empty((B, S, D_OUT), dtype=np.float32)
    for c in range(N_CORES):
        b, m = divmod(c, 2)
        co = results[c]["out"]
        for s in range(N_SLOTS):
            i = 2 * s + 1 - m
            out[b, QS * i:QS * i + QS, :] = co[QS * s:QS * s + QS, :]
    return out


def _run(inputs, trace=False):
    from concourse.bass_utils import run_bass_kernel_spmd
    if "nc" not in _STATE:
        _STATE["nc"] = _build_program()
    res = run_bass_kernel_spmd(_STATE["nc"], _host_inputs(inputs),
                               list(range(N_CORES)), trace=trace)
    return _assemble(res.results), res


def kernel(**inputs):
    out, _ = _run(inputs, trace=False)
    return out
